# revision 1
# baseline (speedup 1.0000x reference)
"""LoFTR LocalFeatureTransformer on 8 NeuronCores (Trainium2 via PJRT).

Sharding: data-parallel over the 8 independent sequences (4 batches x
{feat0, feat1}), one sequence per NeuronCore. Every layer applies the same
weights to all 8 sequences, so self-attention layers are fully local.
Cross-attention layers only need the partner sequence's linear-attention
KV statistics ([H,D,D] + [H,D] ~ 33KB), exchanged with a pairwise
ppermute instead of moving the full 4.7MB sequence.

Device i holds: i even -> feat0[i//2], i odd -> feat1[i//2]; partner = i^1.

Compilation notes (neuronx-cc in this container):
- Fusing all 8 layers into one module, or passing weights as pmap
  parameters, trips an internal tensorizer assertion (MaskPropagation /
  "Need to split to perfect loopnest").
- One pmap module per layer with the layer's weights baked in as
  constants compiles cleanly, so that is what we do. Activations stay
  device-resident between the 8 calls.
"""

import functools

import jax
import jax.numpy as jnp
import numpy as np

D_MODEL = 256
NHEAD = 8
HEAD_DIM = D_MODEL // NHEAD
NLAYERS = 8
LAYER_NAMES = ('self', 'cross', 'self', 'cross', 'self', 'cross', 'self', 'cross')
LN_EPS = 1e-5
ATTN_EPS = 1e-6
N_CORES = 8
AXIS = 'c'

_SWAP = [(i, i ^ 1) for i in range(N_CORES)]


def _elu1(x):
    # elu(x) + 1 == max(x, 0) + exp(min(x, 0)); select-free for the tensorizer
    return jnp.maximum(x, 0.0) + jnp.exp(jnp.minimum(x, 0.0))


def _layernorm(x, g, b):
    mu = jnp.mean(x, axis=-1, keepdims=True)
    var = jnp.mean(jnp.square(x - mu), axis=-1, keepdims=True)
    return (x - mu) * jax.lax.rsqrt(var + LN_EPS) * g + b


def _make_layer(Wq, Wk, Wv, Wm, W1, W2, g1, b1, g2, b2, cross):
    Wq, Wk, Wv, Wm = map(jnp.asarray, (Wq, Wk, Wv, Wm))
    W1a = jnp.asarray(W1[:D_MODEL])
    W1b = jnp.asarray(W1[D_MODEL:])
    W2 = jnp.asarray(W2)
    g1, b1, g2, b2 = map(jnp.asarray, (g1, b1, g2, b2))

    def f(x):
        L = x.shape[0]
        q = x @ Wq
        k = x @ Wk
        v = x @ Wv
        Q = _elu1(q).reshape(L, NHEAD, HEAD_DIM).transpose(1, 0, 2)   # [H,L,D]
        K = _elu1(k).reshape(L, NHEAD, HEAD_DIM).transpose(1, 0, 2)
        Vn = (v / L).reshape(L, NHEAD, HEAD_DIM).transpose(1, 0, 2)
        KV = jnp.matmul(K.transpose(0, 2, 1), Vn)                     # [H,D,D]
        Ksum = K.sum(axis=1)                                          # [H,D]
        if cross:
            KV = jax.lax.ppermute(KV, AXIS, _SWAP)
            Ksum = jax.lax.ppermute(Ksum, AXIS, _SWAP)
        den = jnp.matmul(Q, Ksum[:, :, None])                         # [H,L,1]
        Z = 1.0 / (den + ATTN_EPS)
        msg = jnp.matmul(Q, KV) * Z * L                               # [H,L,D]
        msg = msg.transpose(1, 0, 2).reshape(L, D_MODEL)
        msg = _layernorm(msg @ Wm, g1, b1)
        h = jax.nn.relu(x @ W1a + msg @ W1b) @ W2
        h = _layernorm(h, g2, b2)
        return x + h

    return f


def _build_layers(ws):
    """Four pmap modules, each fusing one (self, cross) layer pair.

    2-layer fusion compiles cleanly; full 8-layer fusion trips the
    tensorizer assertion noted above.
    """
    fns = []
    for i in range(0, NLAYERS, 2):
        fa = _make_layer(*(w[i] for w in ws),
                         cross=(LAYER_NAMES[i] == 'cross'))
        fb = _make_layer(*(w[i + 1] for w in ws),
                         cross=(LAYER_NAMES[i + 1] == 'cross'))
        fns.append(jax.pmap(lambda x, fa=fa, fb=fb: fb(fa(x)),
                            axis_name=AXIS))
    return fns


_cache = {}


def kernel(feat0, feat1, Wq, Wk, Wv, Wm, W1, W2, g1, b1, g2, b2):
    feat0 = np.asarray(feat0, dtype=np.float32)
    feat1 = np.asarray(feat1, dtype=np.float32)
    N, L, C = feat0.shape

    key = id(Wq)
    if key not in _cache:
        _cache.clear()
        ws = [np.asarray(w, dtype=np.float32)
              for w in (Wq, Wk, Wv, Wm, W1, W2, g1, b1, g2, b2)]
        _cache[key] = _build_layers(ws)
    layers = _cache[key]

    seqs = np.empty((N_CORES, L, C), dtype=np.float32)
    seqs[0::2] = feat0
    seqs[1::2] = feat1
    x = jax.device_put_sharded(list(seqs), jax.devices()[:N_CORES])

    for f in layers:
        x = f(x)

    out = np.asarray(x)
    return out[0::2].copy(), out[1::2].copy()



# revision 16
# speedup vs baseline: 2.5050x; 2.5050x over previous
"""LoFTR LocalFeatureTransformer — hand-written Bass/Tile kernel for 8 NeuronCores.

Sharding: data-parallel over the 8 sequences (4 batches x {feat0, feat1}),
one sequence per core.  Core i holds feat0[i//2] (i even) / feat1[i//2]
(i odd); partner = i ^ 1.  Self-attention layers are fully local; cross
layers exchange only the linear-attention statistics (KV [H,D,D] + Ksum
[H,D] ~ 132 KB fp32) with the partner core via a pairwise AllReduce and
recover the partner's stats as (sum - own).

Per-core kernel (all 8 layers in one NEFF, activations SBUF-resident bf16):
  x kept in both layouts: x_norm [l, c] and xT [c, l] (T-interleaved form
  produced by the SBUF->SBUF DMA xbar transpose).
  Per layer:
    K~T/V~T = proj via weight-stationary matmuls (outputs transposed);
      elu1(k) = exp(min(k,0)) + max(k,0) = min(exp(k),1) + relu(k)
      computed with ACT Exp + DVE min/max; DMA-transpose to [s, hd] layout.
    stats: KV_m = K~^T V~ (contract over s), Ksum = K~^T 1 (pad rows
      excluded via a truncated ones column).  Cross layers AllReduce the
      (KV, Ksum) blob over core pairs and use partner = sum - own.
    Q~T likewise (kept transposed);  den8[h, l] = Ksum-matmul on Q~T;
      Z = 1/den broadcast to partitions via a tiny 0/1 matmul;
      Q^ = Q~ * Z  (folds the attention denominator into Q before KV).
    msgT = KVbd-stationary @ Q^T;  y1 = msgT-chunks @ [Wm | rowmean(Wm)]
      (the extra column yields the LN1 mean for free).
    LN1: var from ACT Square+accum of (y1-mu); scale by rsqrt via per-
      partition tensor_scalar; g1/b1 are folded into W1b/bias host-side.
    h1T = W1-stationary @ [xT; tT] with fused bias+relu;  h2 = h1T-chunks
      @ [W2 | rowmean(W2)];  LN2 + residual; g2/b2 applied only if
      nontrivial.  DMA-transpose x_new -> xT for the next layer.

kernel(**inputs) takes the FULL unsharded inputs and returns
(feat0_out, feat1_out) like the reference.
"""

import numpy as np
import ml_dtypes

import concourse.bass as bass
import concourse.bacc as bacc
import concourse.mybir as mybir
import concourse.tile as tile

F32 = mybir.dt.float32
BF16 = mybir.dt.bfloat16
AF = mybir.ActivationFunctionType
OP = mybir.AluOpType

D = 256            # d_model
NH = 8             # heads
HD = 32            # head dim
L_REAL = 4800
NCHUNK = 38        # 4864 / 128
L = NCHUNK * 128   # 4864 padded
NLAYERS = 8
LN_EPS = 1e-5
N_CORES = 8

# l-blocks in chunk units: (chunk0, nchunks); 9 x 512 + 1 x 256 elements
BLOCKS = [(i * 4, 4) for i in range(9)] + [(36, 2)]

# packed weight blob column offsets (per layer, [128, WCOLS] bf16)
OFF_WQ = 0            # [2k, 256]
OFF_WK = 512          # [2k, 256]
OFF_WV = 1024         # [2k, 256]
OFF_WM = 1536         # [2k, 257]
OFF_W1 = 2050         # [4k, 512]
OFF_W2 = 4098         # [4k, 257]
WCOLS = 5126


def _build_module(ncores, apply_g2, apply_b2):
    nc = bacc.Bacc(
        "TRN2", target_bir_lowering=False, debug=False,
        enable_asserts=False, num_devices=ncores,
    )
    xn_in = nc.dram_tensor("xn_in", [128, NCHUNK * 256], BF16, kind="ExternalInput").ap()
    xt_in = nc.dram_tensor("xt_in", [128, 2 * L], BF16, kind="ExternalInput").ap()
    wts_in = nc.dram_tensor("wts", [NLAYERS, 128, WCOLS], BF16, kind="ExternalInput").ap()
    b1w_in = nc.dram_tensor("b1w", [NLAYERS, 128, 4], F32, kind="ExternalInput").ap()
    g2b2_in = None
    if apply_g2 or apply_b2:
        g2b2_in = nc.dram_tensor(
            "g2b2", [128, NLAYERS * 2 * 256], F32, kind="ExternalInput").ap()
    b8_in = nc.dram_tensor("b8", [8, 256], BF16, kind="ExternalInput").ap()
    ones_in = nc.dram_tensor("ones_t", [128, 2], BF16, kind="ExternalInput").ap()
    y_out = nc.dram_tensor("y", [128, NCHUNK * 256], BF16, kind="ExternalOutput").ap()

    groups = [[2 * i, 2 * i + 1] for i in range(ncores // 2)] if ncores > 1 else [[0]]

    with tile.TileContext(nc) as tc:
        with (
            tc.tile_pool(name="persist", bufs=1) as pp,
            tc.tile_pool(name="big", bufs=1) as bigp,
            tc.tile_pool(name="h1p", bufs=2) as h1p,
            tc.tile_pool(name="esc", bufs=4) as escp,
            tc.tile_pool(name="wp", bufs=2) as wp,
            tc.tile_pool(name="psum", bufs=5, space="PSUM") as psp,
            tc.tile_pool(name="pskv", bufs=2, space="PSUM") as pskv,
            tc.tile_pool(name="psks", bufs=1, space="PSUM") as psks,
            tc.tile_pool(name="dram", bufs=8, space="DRAM") as dramp,
        ):
            # ---------- persistent state ----------
            xn = pp.tile([128, NCHUNK, 256], BF16, tag="xn")
            xt = pp.tile([128, 76, 128], BF16, tag="xt")       # T-form: [p,2c+k,j]
            kvbd = pp.tile([128, 2, 128], BF16, tag="kvbd")
            a8 = pp.tile([128, 2, 8], BF16, tag="a8")
            b8t = pp.tile([8, 2, 128], BF16, tag="b8t")
            onest = pp.tile([128, 2], BF16, tag="onest")
            z8 = pp.tile([8, L], BF16, tag="z8")
            ss1 = pp.tile([128, NCHUNK], F32, tag="ss1")
            ss2 = pp.tile([128, NCHUNK], F32, tag="ss2")
            rs1 = pp.tile([128, NCHUNK], F32, tag="rs1")
            rs2 = pp.tile([128, NCHUNK], F32, tag="rs2")
            lnt = pp.tile([128, NCHUNK], F32, tag="lnt")
            junk = pp.tile([128, 256], BF16, tag="junk")
            epst = pp.tile([128, 1], F32, tag="epst")
            blob = pp.tile([128, 258], F32, tag="blob")
            bsum = pp.tile([128, 258], F32, tag="bsum")
            g2b2 = None
            if apply_g2 or apply_b2:
                # row-replicated host-side: g2b2[p, layer, which, c] = val[layer, which, c]
                g2b2 = pp.tile([128, NLAYERS, 2, 256], F32, tag="g2b2")

            nc.sync.dma_start(xn[:], xn_in.rearrange("p (c d) -> p c d", d=256))
            nc.sync.dma_start(xt[:], xt_in.rearrange("p (t j) -> p t j", j=128))
            nc.sync.dma_start(b8t[:], b8_in.rearrange("r (k j) -> r k j", j=128))
            nc.sync.dma_start(onest[:], ones_in)
            if g2b2 is not None:
                nc.sync.dma_start(
                    g2b2[:], g2b2_in.rearrange("p (l w c) -> p l w c", w=2, c=256))
            nc.vector.memset(epst[:], LN_EPS)
            nc.vector.memset(kvbd[:], 0.0)
            nc.vector.memset(a8[:], 0.0)

            def xt_like_rhs(t, k, cb0, nbc):
                v = t.rearrange("p (c two) j -> p c two j", two=2)
                return v[:, cb0:cb0 + nbc, k, :]

            for layer in range(NLAYERS):
                cross = (layer % 2 == 1)
                wt = wp.tile([128, WCOLS], BF16, tag="wt")
                nc.sync.dma_start(wt[:], wts_in[layer])
                b1w = wp.tile([128, 4], F32, tag="b1w")
                nc.sync.dma_start(b1w[:], b1w_in[layer])

                def wl(off, k, m, width=128, per_k=256):
                    return wt[:, off + k * per_k + m * width: off + k * per_k + (m + 1) * width]

                # ---------- A: K~T, V~T (transposed) + DMA transpose ----------
                ktT = bigp.tile([128, 2, L], BF16, tag="bigA")
                vtT = bigp.tile([128, 2, L], BF16, tag="bigB")
                for m in range(2):
                    for cb0, nbc in BLOCKS:
                        bw = nbc * 128
                        lsl = slice(cb0 * 128, cb0 * 128 + bw)
                        ps = psp.tile([128, 512], F32, tag="blk", name="psblk")[:, :bw]
                        for k in range(2):
                            nc.tensor.matmul(ps, wl(OFF_WK, k, m), xt_like_rhs(xt, k, cb0, nbc),
                                             start=(k == 0), stop=(k == 1))
                        e = escp.tile([128, 512], BF16, tag="esc", name="esc")[:, :bw]
                        nc.scalar.activation(e, ps, AF.Exp)
                        nc.vector.tensor_scalar_min(e, e, 1.0)
                        nc.vector.scalar_tensor_tensor(
                            ktT[:, m, lsl], ps, 0.0, e, OP.max, OP.add)
                        ps2 = psp.tile([128, 512], F32, tag="blk", name="psblk")[:, :bw]
                        for k in range(2):
                            nc.tensor.matmul(ps2, wl(OFF_WV, k, m), xt_like_rhs(xt, k, cb0, nbc),
                                             start=(k == 0), stop=(k == 1))
                        nc.vector.tensor_copy(vtT[:, m, lsl], ps2)
                ktn = bigp.tile([128, 76, 128], BF16, tag="bigD")
                vtn = bigp.tile([128, 76, 128], BF16, tag="bigC")
                nc.sync.dma_start_transpose(ktn[:], ktT[:])
                nc.sync.dma_start_transpose(vtn[:], vtT[:])

                # ---------- B: stats ----------
                kv0 = pskv.tile([128, 128], F32, tag="kv")
                kv1 = pskv.tile([128, 128], F32, tag="kv")
                ksm = psks.tile([128, 2], F32, tag="ks")
                kvp = [kv0, kv1]
                ktn_r = ktn.rearrange("p (m c) j -> p m c j", m=2)
                vtn_r = vtn.rearrange("p (m c) j -> p m c j", m=2)
                for c in range(NCHUNK):
                    onecol = onest[:, 1:2] if c == NCHUNK - 1 else onest[:, 0:1]
                    for m in range(2):
                        lhsT = ktn_r[:, m, c, :]
                        nc.tensor.matmul(kvp[m][:], lhsT, vtn_r[:, m, c, :],
                                         start=(c == 0), stop=(c == NCHUNK - 1))
                        nc.tensor.matmul(ksm[:, m:m + 1], lhsT, onecol,
                                         start=(c == 0), stop=(c == NCHUNK - 1),
                                         skip_group_check=True)

                if cross:
                    nc.vector.tensor_copy(blob[:, 0:128], kv0[:])
                    nc.vector.tensor_copy(blob[:, 128:256], kv1[:])
                    nc.vector.tensor_copy(blob[:, 256:258], ksm[:])
                    din = dramp.tile([128, 258], F32, tag="din")
                    dout = dramp.tile([128, 258], F32, tag="dout")
                    nc.sync.dma_start(din[:], blob[:])
                    nc.gpsimd.collective_compute(
                        "AllReduce", OP.add, replica_groups=groups,
                        ins=[din.opt()], outs=[dout.opt()])
                    nc.sync.dma_start(bsum[:], dout[:])
                    if ncores > 1:
                        nc.vector.tensor_tensor(
                            bsum[:], bsum[:], blob[:], OP.subtract)
                    kvsrc = [bsum[:, 0:128], bsum[:, 128:256]]
                    kssrc = bsum[:, 256:258]
                else:
                    kvsrc = [kv0[:], kv1[:]]
                    kssrc = ksm[:]

                for m in range(2):
                    for h in range(4):
                        r = 32 * h
                        nc.vector.tensor_copy(
                            kvbd[r:r + 32, m, r:r + 32], kvsrc[m][r:r + 32, r:r + 32])
                for h8 in range(8):
                    k, hh = h8 // 4, h8 % 4
                    nc.vector.tensor_copy(
                        a8[32 * hh:32 * hh + 32, k, h8:h8 + 1],
                        kssrc[32 * hh:32 * hh + 32, k:k + 1])

                # ---------- C: Q~T ----------
                qt = bigp.tile([128, 2, L], BF16, tag="bigA")
                for m in range(2):
                    for cb0, nbc in BLOCKS:
                        bw = nbc * 128
                        lsl = slice(cb0 * 128, cb0 * 128 + bw)
                        ps = psp.tile([128, 512], F32, tag="blk", name="psblk")[:, :bw]
                        for k in range(2):
                            nc.tensor.matmul(ps, wl(OFF_WQ, k, m), xt_like_rhs(xt, k, cb0, nbc),
                                             start=(k == 0), stop=(k == 1))
                        e = escp.tile([128, 512], BF16, tag="esc", name="esc")[:, :bw]
                        nc.scalar.activation(e, ps, AF.Exp)
                        nc.vector.tensor_scalar_min(e, e, 1.0)
                        nc.vector.scalar_tensor_tensor(
                            qt[:, m, lsl], ps, 0.0, e, OP.max, OP.add)

                # ---------- D: den8 / Z / Q^ / msgT ----------
                for cb0, nbc in BLOCKS:
                    bw = nbc * 128
                    lsl = slice(cb0 * 128, cb0 * 128 + bw)
                    psd = psp.tile([8, 512], F32, tag="blk", name="psd8")[:, :bw]
                    for k in range(2):
                        nc.tensor.matmul(psd, a8[:, k, :], qt[:, k, lsl],
                                         start=(k == 0), stop=(k == 1))
                    with nc.allow_low_precision(reason="attention denominator in bf16 is within tolerance"):
                        nc.vector.reciprocal(z8[:, lsl], psd)
                for k in range(2):
                    for cb0, nbc in BLOCKS:
                        bw = nbc * 128
                        lsl = slice(cb0 * 128, cb0 * 128 + bw)
                        psz = psp.tile([128, 512], F32, tag="blk", name="psblk")[:, :bw]
                        nc.tensor.matmul(psz, b8t[:, k, :], z8[:, lsl],
                                         start=True, stop=True)
                        nc.vector.tensor_tensor(qt[:, k, lsl], qt[:, k, lsl], psz, OP.mult)
                msgts = bigp.tile([128, 2, L], BF16, tag="bigE")
                for m in range(2):
                    for cb0, nbc in BLOCKS:
                        bw = nbc * 128
                        lsl = slice(cb0 * 128, cb0 * 128 + bw)
                        psm = psp.tile([128, 512], F32, tag="blk", name="psblk")[:, :bw]
                        nc.tensor.matmul(psm, kvbd[:, m, :], qt[:, m, lsl],
                                         start=True, stop=True)
                        if m == 0:
                            nc.vector.tensor_copy(msgts[:, m, lsl], psm)
                        else:
                            nc.scalar.activation(msgts[:, m, lsl], psm, AF.Copy)

                # ---------- E: Wm + LN1 -> t, tT ----------
                w1t = bigp.tile([128, NCHUNK, 256], BF16, tag="bigC")
                for c in range(NCHUNK):
                    psy = psp.tile([128, 512], F32, tag="blk", name="psy")[:, :257]
                    for k in range(2):
                        nc.tensor.matmul(psy, msgts[:, k, 128 * c:128 * (c + 1)],
                                         wt[:, OFF_WM + k * 257: OFF_WM + (k + 1) * 257],
                                         start=(k == 0), stop=(k == 1))
                    nc.vector.tensor_scalar_sub(w1t[:, c, :], psy[:, :256], psy[:, 256:257])
                    nc.scalar.activation(junk[:], w1t[:, c, :], AF.Square,
                                         accum_out=ss1[:, c:c + 1])
                nc.scalar.activation(lnt[:], ss1[:], AF.Ln, bias=epst[:], scale=1.0 / 256)
                nc.scalar.activation(rs1[:], lnt[:], AF.Exp, scale=-0.5)
                t_ = bigp.tile([128, NCHUNK, 256], BF16, tag="bigA")
                for c in range(NCHUNK):
                    nc.vector.tensor_scalar_mul(t_[:, c, :], w1t[:, c, :], rs1[:, c:c + 1])
                tt = bigp.tile([128, 76, 128], BF16, tag="bigD")
                nc.sync.dma_start_transpose(tt[:], t_[:])

                # ---------- F: h1 (+bias+relu) and h2 + LN2, interleaved ----------
                w2t = bigp.tile([128, NCHUNK, 256], BF16, tag="bigE")
                for cb0, nbc in BLOCKS:
                    bw = nbc * 128
                    h1b = h1p.tile([128, 4, 512], BF16, tag="h1b")
                    for m in range(4):
                        psh = psp.tile([128, 512], F32, tag="blk", name="psblk")[:, :bw]
                        for k in range(2):
                            nc.tensor.matmul(psh, wl(OFF_W1, k, m, per_k=512),
                                             xt_like_rhs(xt, k, cb0, nbc),
                                             start=(k == 0), stop=False)
                        for k in range(2):
                            nc.tensor.matmul(psh, wl(OFF_W1, k + 2, m, per_k=512),
                                             xt_like_rhs(tt, k, cb0, nbc),
                                             start=False, stop=(k == 1))
                        if m < 2:
                            nc.scalar.activation(h1b[:, m, :bw], psh, AF.Relu,
                                                 bias=b1w[:, m:m + 1])
                        else:
                            nc.vector.tensor_scalar(h1b[:, m, :bw], psh, b1w[:, m:m + 1],
                                                    0.0, OP.add, OP.max)
                    for cc in range(nbc):
                        c = cb0 + cc
                        psh2 = psp.tile([128, 512], F32, tag="blk", name="psy")[:, :257]
                        for k in range(4):
                            nc.tensor.matmul(psh2, h1b[:, k, 128 * cc:128 * (cc + 1)],
                                             wt[:, OFF_W2 + k * 257: OFF_W2 + (k + 1) * 257],
                                             start=(k == 0), stop=(k == 3))
                        nc.vector.tensor_scalar_sub(w2t[:, c, :], psh2[:, :256],
                                                    psh2[:, 256:257])
                        nc.vector.scalar_tensor_tensor(
                            junk[:], w2t[:, c, :], 1.0, w2t[:, c, :],
                            OP.bypass, OP.mult, accum_out=ss2[:, c:c + 1])
                nc.scalar.activation(lnt[:], ss2[:], AF.Ln, bias=epst[:], scale=1.0 / 256)
                nc.scalar.activation(rs2[:], lnt[:], AF.Exp, scale=-0.5)
                u = bigp.tile([128, NCHUNK, 256], BF16, tag="bigB")
                for c in range(NCHUNK):
                    nc.vector.tensor_scalar_mul(u[:, c, :], w2t[:, c, :], rs2[:, c:c + 1])
                if apply_g2:
                    gv = g2b2[:, layer, 0, None, :].to_broadcast((128, NCHUNK, 256))
                    nc.vector.tensor_tensor(u[:], u[:], gv, OP.mult)
                if apply_b2:
                    bv = g2b2[:, layer, 1, None, :].to_broadcast((128, NCHUNK, 256))
                    nc.vector.tensor_tensor(u[:], u[:], bv, OP.add)
                nc.vector.tensor_tensor(xn[:], u[:], xn[:], OP.add)
                nc.vector.memset(xn[64:128, NCHUNK - 1, :], 0.0)
                nc.sync.dma_start_transpose(xt[:], xn[:])

            nc.sync.dma_start(y_out.rearrange("p (c d) -> p c d", d=256), xn[:])

    nc.compile()
    return nc


# NOTE on apply_g2/b2: the LN gamma/beta of the second layernorm cannot be
# folded into weights; they are applied with extra DVE passes only when they
# are not the trivial (1, 0).  g1/b1 are always applied exactly (folded into
# W1b / the h1 bias host-side), as are all other parameters.


def _pack_inputs(feat_seq):
    """Host-side packing. feat_seq: [L_REAL, 256] fp32 for one core."""
    bf = ml_dtypes.bfloat16
    x = np.zeros((L, D), np.float32)
    x[:L_REAL] = feat_seq
    xn = np.ascontiguousarray(
        x.reshape(NCHUNK, 128, D).transpose(1, 0, 2).reshape(128, NCHUNK * D)
    ).astype(bf)
    # xt T-form: xt[p, 2c+k, j] = x[c*128+j, k*128+p]
    xr = x.reshape(NCHUNK, 128, 2, 128)          # [c, j, k, p]
    xt = np.ascontiguousarray(
        xr.transpose(3, 0, 2, 1).reshape(128, 2 * L)
    ).astype(bf)
    return xn, xt


def _pack_weights(Wq, Wk, Wv, Wm, W1, W2, g1, b1, g2, b2):
    bf = ml_dtypes.bfloat16
    wts = np.zeros((NLAYERS, 128, WCOLS), np.float32)
    b1w = np.zeros((NLAYERS, 128, 4), np.float32)
    g2b2 = np.zeros((NLAYERS, 2, 256), np.float32)
    for i in range(NLAYERS):
        wq = Wq[i].reshape(2, 128, 256).transpose(1, 0, 2).reshape(128, 512)
        wk = Wk[i].reshape(2, 128, 256).transpose(1, 0, 2).reshape(128, 512)
        wv = Wv[i].reshape(2, 128, 256).transpose(1, 0, 2).reshape(128, 512)
        wm_aug = np.concatenate([Wm[i], Wm[i].mean(axis=1, keepdims=True)], axis=1)
        wm = wm_aug.reshape(2, 128, 257).transpose(1, 0, 2).reshape(128, 514)
        w1cat = np.concatenate([W1[i][:256], g1[i][:, None] * W1[i][256:]], axis=0)
        w1 = w1cat.reshape(4, 128, 512).transpose(1, 0, 2).reshape(128, 2048)
        w2_aug = np.concatenate([W2[i], W2[i].mean(axis=1, keepdims=True)], axis=1)
        w2 = w2_aug.reshape(4, 128, 257).transpose(1, 0, 2).reshape(128, 1028)
        wts[i, :, OFF_WQ:OFF_WQ + 512] = wq
        wts[i, :, OFF_WK:OFF_WK + 512] = wk
        wts[i, :, OFF_WV:OFF_WV + 512] = wv
        wts[i, :, OFF_WM:OFF_WM + 514] = wm
        wts[i, :, OFF_W1:OFF_W1 + 2048] = w1
        wts[i, :, OFF_W2:OFF_W2 + 1028] = w2
        b1w[i] = (b1[i] @ W1[i][256:]).reshape(4, 128).T
        g2b2[i, 0] = g2[i]
        g2b2[i, 1] = b2[i]
    b8 = np.zeros((8, 256), np.float32)
    for k in range(2):
        for j in range(128):
            b8[4 * k + j // 32, k * 128 + j] = 1.0
    ones_t = np.zeros((128, 2), np.float32)
    ones_t[:, 0] = 1.0
    ones_t[:64, 1] = 1.0
    return (wts.astype(bf), b1w, g2b2, b8.astype(bf), ones_t.astype(bf))


class _Runner:
    """Builds the module once and keeps a cached jitted executable
    (mirrors bass2jax.run_bass_via_pjrt's multi-core path, without donation
    so device-resident inputs can be reused across timed iterations)."""

    def __init__(self, ncores, apply_g2, apply_b2):
        import jax
        from jax.sharding import Mesh, PartitionSpec
        from jax.experimental.shard_map import shard_map
        from concourse import bass2jax

        self.ncores = ncores
        nc = _build_module(ncores, apply_g2, apply_b2)
        self.nc = nc
        bass2jax.install_neuronx_cc_hook()

        part_name = nc.partition_id_tensor.name if nc.partition_id_tensor else None
        in_names = []
        out_names = []
        out_avals = []
        zero_outs = []
        for alloc in nc.m.functions[0].allocations:
            if not isinstance(alloc, mybir.MemoryLocationSet):
                continue
            name = alloc.memorylocations[0].name
            if alloc.kind == "ExternalInput":
                if name != part_name:
                    in_names.append(name)
            elif alloc.kind == "ExternalOutput":
                out_names.append(name)
                shape = tuple(alloc.tensor_shape)
                dtype = mybir.dt.np(alloc.dtype)
                out_avals.append(jax.core.ShapedArray(shape, dtype))
                zero_outs.append(np.zeros(shape, dtype))
        self.in_names = in_names
        self.out_names = out_names
        n_params = len(in_names)
        all_names = in_names + out_names
        if part_name is not None:
            all_names = all_names + [part_name]

        def _body(*args):
            operands = list(args)
            if part_name is not None:
                operands.append(bass2jax.partition_id_tensor())
            outs = bass2jax._bass_exec_p.bind(
                *operands,
                out_avals=tuple(out_avals),
                in_names=tuple(all_names),
                out_names=tuple(out_names),
                lowering_input_output_aliases=(),
                sim_require_finite=False,
                sim_require_nnan=False,
                nc=nc,
            )
            return tuple(outs)

        devices = jax.devices()[:ncores]
        assert len(devices) == ncores
        mesh = Mesh(np.asarray(devices), ("core",))
        self.mesh = mesh
        nin = n_params + len(zero_outs)
        self.sharded = jax.jit(
            shard_map(_body, mesh=mesh,
                      in_specs=(PartitionSpec("core"),) * nin,
                      out_specs=(PartitionSpec("core"),) * len(out_names),
                      check_rep=False),
            keep_unused=True,
        )
        self.zero_outs = zero_outs
        self.jax = jax

    def concat_inputs(self, in_maps):
        outs = [
            np.concatenate([np.asarray(m[name]) for m in in_maps], axis=0)
            for name in self.in_names
        ]
        outs += [
            np.zeros((self.ncores * z.shape[0], *z.shape[1:]), z.dtype)
            for z in self.zero_outs
        ]
        return outs

    def run(self, in_maps):
        args = self.concat_inputs(in_maps)
        out_arrs = self.sharded(*args)
        res = []
        for c in range(self.ncores):
            d = {}
            for i, name in enumerate(self.out_names):
                full = np.asarray(out_arrs[i])
                d[name] = full.reshape(
                    self.ncores, full.shape[0] // self.ncores, *full.shape[1:])[c]
            res.append(d)
        return res


_cache = {}


def _get_runner(ncores, apply_g2, apply_b2):
    key = (ncores, apply_g2, apply_b2)
    if key not in _cache:
        _cache[key] = _Runner(ncores, apply_g2, apply_b2)
    return _cache[key]


_wcache = {}


def prepare(feat0, feat1, Wq, Wk, Wv, Wm, W1, W2, g1, b1, g2, b2):
    """Pack inputs and return (runner, in_maps) — shared by kernel() and test.py."""
    feat0 = np.asarray(feat0, np.float32)
    feat1 = np.asarray(feat1, np.float32)
    N = feat0.shape[0]
    assert N * 2 == N_CORES

    wkey = id(Wq)
    if wkey not in _wcache:
        _wcache.clear()
        args = [np.asarray(a, np.float32) for a in
                (Wq, Wk, Wv, Wm, W1, W2, g1, b1, g2, b2)]
        packed = _pack_weights(*args)
        apply_g2 = not np.allclose(args[8], 1.0)
        apply_b2 = not np.allclose(args[9], 0.0)
        _wcache[wkey] = (packed, apply_g2, apply_b2)
    (wts, b1w, g2b2, b8, ones_t), apply_g2, apply_b2 = _wcache[wkey]

    runner = _get_runner(N_CORES, apply_g2, apply_b2)

    g2b2_rep = None
    if apply_g2 or apply_b2:
        g2b2_rep = np.ascontiguousarray(np.broadcast_to(
            g2b2.reshape(1, NLAYERS * 2 * 256), (128, NLAYERS * 2 * 256)))

    in_maps = []
    for core in range(N_CORES):
        seq = feat0[core // 2] if core % 2 == 0 else feat1[core // 2]
        xn, xt = _pack_inputs(seq)
        m = dict(xn_in=xn, xt_in=xt, wts=wts, b1w=b1w, b8=b8, ones_t=ones_t)
        if g2b2_rep is not None:
            m["g2b2"] = g2b2_rep
        in_maps.append(m)
    return runner, in_maps


def kernel(feat0, feat1, Wq, Wk, Wv, Wm, W1, W2, g1, b1, g2, b2):
    feat0 = np.asarray(feat0, np.float32)
    feat1 = np.asarray(feat1, np.float32)
    N = feat0.shape[0]
    runner, in_maps = prepare(feat0, feat1, Wq, Wk, Wv, Wm, W1, W2,
                              g1, b1, g2, b2)
    res = runner.run(in_maps)

    out0 = np.empty((N, L_REAL, D), np.float32)
    out1 = np.empty((N, L_REAL, D), np.float32)
    for core in range(N_CORES):
        y = np.asarray(res[core]["y"], np.float32)  # [128, NCHUNK*256]
        xfull = y.reshape(128, NCHUNK, D).transpose(1, 0, 2).reshape(L, D)[:L_REAL]
        if core % 2 == 0:
            out0[core // 2] = xfull
        else:
            out1[core // 2] = xfull
    return out0, out1


# revision 17
# speedup vs baseline: 3.9194x; 1.5646x over previous
"""LoFTR LocalFeatureTransformer — hand-written Bass/Tile kernel for 8 NeuronCores.

Sharding: data-parallel over the 8 sequences (4 batches x {feat0, feat1}),
one sequence per core.  Core i holds feat0[i//2] (i even) / feat1[i//2]
(i odd); partner = i ^ 1.  Self-attention layers are fully local; cross
layers exchange only the linear-attention statistics (KV [H,D,D] + Ksum
[H,D] ~ 132 KB fp32) with the partner core via a pairwise AllReduce and
recover the partner's stats as (sum - own).

Per-core kernel (all 8 layers in one NEFF, activations SBUF-resident bf16):
  x kept in both layouts: x_norm [l, c] and xT [c, l] (T-interleaved form
  produced by the SBUF->SBUF DMA xbar transpose).
  Per layer:
    K~T/V~T = proj via weight-stationary matmuls (outputs transposed);
      elu1(k) = exp(min(k,0)) + max(k,0) = min(exp(k),1) + relu(k)
      computed with ACT Exp + DVE min/max; DMA-transpose to [s, hd] layout.
    stats: KV_m = K~^T V~ (contract over s), Ksum = K~^T 1 (pad rows
      excluded via a truncated ones column).  Cross layers AllReduce the
      (KV, Ksum) blob over core pairs and use partner = sum - own.
    Q~T likewise (kept transposed);  den8[h, l] = Ksum-matmul on Q~T;
      Z = 1/den broadcast to partitions via a tiny 0/1 matmul;
      Q^ = Q~ * Z  (folds the attention denominator into Q before KV).
    msgT = KVbd-stationary @ Q^T;  y1 = msgT-chunks @ [Wm | rowmean(Wm)]
      (the extra column yields the LN1 mean for free).
    LN1: var from ACT Square+accum of (y1-mu); scale by rsqrt via per-
      partition tensor_scalar; g1/b1 are folded into W1b/bias host-side.
    h1T = W1-stationary @ [xT; tT] with fused bias+relu;  h2 = h1T-chunks
      @ [W2 | rowmean(W2)];  LN2 + residual; g2/b2 applied only if
      nontrivial.  DMA-transpose x_new -> xT for the next layer.

kernel(**inputs) takes the FULL unsharded inputs and returns
(feat0_out, feat1_out) like the reference.
"""

import numpy as np
import ml_dtypes

import concourse.bass as bass
import concourse.bacc as bacc
import concourse.mybir as mybir
import concourse.tile as tile

F32 = mybir.dt.float32
BF16 = mybir.dt.bfloat16
AF = mybir.ActivationFunctionType
OP = mybir.AluOpType

D = 256            # d_model
NH = 8             # heads
HD = 32            # head dim
L_REAL = 4800
NCHUNK = 38        # 4864 / 128
L = NCHUNK * 128   # 4864 padded
NLAYERS = 8
LN_EPS = 1e-5
N_CORES = 8

# l-blocks in chunk units: (chunk0, nchunks); 9 x 512 + 1 x 256 elements
BLOCKS = [(i * 4, 4) for i in range(9)] + [(36, 2)]

# packed weight blob column offsets (per layer, [128, WCOLS] bf16)
OFF_WQ = 0            # [2k, 256]
OFF_WK = 512          # [2k, 256]
OFF_WV = 1024         # [2k, 256]
OFF_WM = 1536         # [2k, 257]
OFF_W1 = 2050         # [4k, 512]
OFF_W2 = 4098         # [4k, 257]
WCOLS = 5126


def _build_module(ncores, apply_g2, apply_b2, no_cc=False):
    nc = bacc.Bacc(
        "TRN2", target_bir_lowering=False, debug=False,
        enable_asserts=False, num_devices=ncores,
    )
    xn_in = nc.dram_tensor("xn_in", [128, NCHUNK * 256], BF16, kind="ExternalInput").ap()
    xt_in = nc.dram_tensor("xt_in", [128, 2 * L], BF16, kind="ExternalInput").ap()
    wts_in = nc.dram_tensor("wts", [NLAYERS, 128, WCOLS], BF16, kind="ExternalInput").ap()
    b1w_in = nc.dram_tensor("b1w", [NLAYERS, 128, 4], F32, kind="ExternalInput").ap()
    g2b2_in = None
    if apply_g2 or apply_b2:
        g2b2_in = nc.dram_tensor(
            "g2b2", [128, NLAYERS * 2 * 256], F32, kind="ExternalInput").ap()
    b8_in = nc.dram_tensor("b8", [8, 256], BF16, kind="ExternalInput").ap()
    ones_in = nc.dram_tensor("ones_t", [128, 2], BF16, kind="ExternalInput").ap()
    y_out = nc.dram_tensor("y", [128, NCHUNK * 256], BF16, kind="ExternalOutput").ap()

    groups = [[2 * i, 2 * i + 1] for i in range(ncores // 2)] if ncores > 1 else [[0]]

    with tile.TileContext(nc) as tc:
        with (
            tc.tile_pool(name="persist", bufs=1) as pp,
            tc.tile_pool(name="big", bufs=1) as bigp,
            tc.tile_pool(name="h1p", bufs=2) as h1p,
            tc.tile_pool(name="esc", bufs=4) as escp,
            tc.tile_pool(name="wp", bufs=2) as wp,
            tc.tile_pool(name="psum", bufs=5, space="PSUM") as psp,
            tc.tile_pool(name="pskv", bufs=2, space="PSUM") as pskv,
            tc.tile_pool(name="psks", bufs=1, space="PSUM") as psks,
            tc.tile_pool(name="dram", bufs=8, space="DRAM") as dramp,
        ):
            # ---------- persistent state ----------
            xn = pp.tile([128, NCHUNK, 256], BF16, tag="xn")
            xt = pp.tile([128, 76, 128], BF16, tag="xt")       # T-form: [p,2c+k,j]
            kvbd = pp.tile([128, 2, 128], BF16, tag="kvbd")
            a8 = pp.tile([128, 2, 8], BF16, tag="a8")
            b8t = pp.tile([8, 2, 128], BF16, tag="b8t")
            onest = pp.tile([128, 2], BF16, tag="onest")
            z8 = pp.tile([8, L], BF16, tag="z8")
            ss1 = pp.tile([128, NCHUNK], F32, tag="ss1")
            ss2 = pp.tile([128, NCHUNK], F32, tag="ss2")
            rs1 = pp.tile([128, NCHUNK], F32, tag="rs1")
            rs2 = pp.tile([128, NCHUNK], F32, tag="rs2")
            lnt = pp.tile([128, NCHUNK], F32, tag="lnt")
            junk = pp.tile([128, 256], BF16, tag="junk")
            epst = pp.tile([128, 1], F32, tag="epst")
            blob = pp.tile([128, 258], F32, tag="blob")
            bsum = pp.tile([128, 258], F32, tag="bsum")
            g2b2 = None
            if apply_g2 or apply_b2:
                # row-replicated host-side: g2b2[p, layer, which, c] = val[layer, which, c]
                g2b2 = pp.tile([128, NLAYERS, 2, 256], F32, tag="g2b2")

            nc.sync.dma_start(xn[:], xn_in.rearrange("p (c d) -> p c d", d=256))
            nc.sync.dma_start(xt[:], xt_in.rearrange("p (t j) -> p t j", j=128))
            nc.sync.dma_start(b8t[:], b8_in.rearrange("r (k j) -> r k j", j=128))
            nc.sync.dma_start(onest[:], ones_in)
            if g2b2 is not None:
                nc.sync.dma_start(
                    g2b2[:], g2b2_in.rearrange("p (l w c) -> p l w c", w=2, c=256))
            nc.vector.memset(epst[:], LN_EPS)
            nc.vector.memset(kvbd[:], 0.0)
            nc.vector.memset(a8[:], 0.0)

            def xt_like_rhs(t, k, cb0, nbc):
                v = t.rearrange("p (c two) j -> p c two j", two=2)
                return v[:, cb0:cb0 + nbc, k, :]

            for layer in range(NLAYERS):
                cross = (layer % 2 == 1)
                wt = wp.tile([128, WCOLS], BF16, tag="wt")
                nc.sync.dma_start(wt[:], wts_in[layer])
                b1w = wp.tile([128, 4], F32, tag="b1w")
                nc.sync.dma_start(b1w[:], b1w_in[layer])

                def wl(off, k, m, width=128, per_k=256):
                    return wt[:, off + k * per_k + m * width: off + k * per_k + (m + 1) * width]

                # ---------- A: K~T, V~T (transposed) + DMA transpose ----------
                ktT = bigp.tile([128, 2, L], BF16, tag="bigA")
                vtT = bigp.tile([128, 2, L], BF16, tag="bigB")
                for m in range(2):
                    for cb0, nbc in BLOCKS:
                        bw = nbc * 128
                        lsl = slice(cb0 * 128, cb0 * 128 + bw)
                        ps = psp.tile([128, 512], F32, tag="blk", name="psblk")[:, :bw]
                        for k in range(2):
                            nc.tensor.matmul(ps, wl(OFF_WK, k, m), xt_like_rhs(xt, k, cb0, nbc),
                                             start=(k == 0), stop=(k == 1))
                        e = escp.tile([128, 512], BF16, tag="esc", name="esc")[:, :bw]
                        nc.scalar.activation(e, ps, AF.Exp)
                        nc.vector.tensor_scalar_min(e, e, 1.0)
                        nc.vector.scalar_tensor_tensor(
                            ktT[:, m, lsl], ps, 0.0, e, OP.max, OP.add)
                        ps2 = psp.tile([128, 512], F32, tag="blk", name="psblk")[:, :bw]
                        for k in range(2):
                            nc.tensor.matmul(ps2, wl(OFF_WV, k, m), xt_like_rhs(xt, k, cb0, nbc),
                                             start=(k == 0), stop=(k == 1))
                        nc.vector.tensor_copy(vtT[:, m, lsl], ps2)
                ktn = bigp.tile([128, 76, 128], BF16, tag="bigD")
                vtn = bigp.tile([128, 76, 128], BF16, tag="bigC")
                nc.sync.dma_start_transpose(ktn[:], ktT[:])
                nc.sync.dma_start_transpose(vtn[:], vtT[:])

                # ---------- B: stats ----------
                kv0 = pskv.tile([128, 128], F32, tag="kv")
                kv1 = pskv.tile([128, 128], F32, tag="kv")
                ksm = psks.tile([128, 2], F32, tag="ks")
                kvp = [kv0, kv1]
                ktn_r = ktn.rearrange("p (m c) j -> p m c j", m=2)
                vtn_r = vtn.rearrange("p (m c) j -> p m c j", m=2)
                for c in range(NCHUNK):
                    onecol = onest[:, 1:2] if c == NCHUNK - 1 else onest[:, 0:1]
                    for m in range(2):
                        lhsT = ktn_r[:, m, c, :]
                        nc.tensor.matmul(kvp[m][:], lhsT, vtn_r[:, m, c, :],
                                         start=(c == 0), stop=(c == NCHUNK - 1))
                        nc.tensor.matmul(ksm[:, m:m + 1], lhsT, onecol,
                                         start=(c == 0), stop=(c == NCHUNK - 1),
                                         skip_group_check=True)

                if cross and not no_cc:
                    nc.vector.tensor_copy(blob[:, 0:128], kv0[:])
                    nc.vector.tensor_copy(blob[:, 128:256], kv1[:])
                    nc.vector.tensor_copy(blob[:, 256:258], ksm[:])
                    din = dramp.tile([128, 258], F32, tag="din")
                    dout = dramp.tile([128, 258], F32, tag="dout")
                    nc.sync.dma_start(din[:], blob[:])
                    nc.gpsimd.collective_compute(
                        "AllReduce", OP.add, replica_groups=groups,
                        ins=[din.opt()], outs=[dout.opt()])
                    nc.sync.dma_start(bsum[:], dout[:])
                    if ncores > 1:
                        nc.vector.tensor_tensor(
                            bsum[:], bsum[:], blob[:], OP.subtract)
                    kvsrc = [bsum[:, 0:128], bsum[:, 128:256]]
                    kssrc = bsum[:, 256:258]
                else:
                    kvsrc = [kv0[:], kv1[:]]
                    kssrc = ksm[:]

                for m in range(2):
                    for h in range(4):
                        r = 32 * h
                        nc.vector.tensor_copy(
                            kvbd[r:r + 32, m, r:r + 32], kvsrc[m][r:r + 32, r:r + 32])
                for h8 in range(8):
                    k, hh = h8 // 4, h8 % 4
                    nc.vector.tensor_copy(
                        a8[32 * hh:32 * hh + 32, k, h8:h8 + 1],
                        kssrc[32 * hh:32 * hh + 32, k:k + 1])

                # ---------- C: Q~T ----------
                qt = bigp.tile([128, 2, L], BF16, tag="bigA")
                for m in range(2):
                    for cb0, nbc in BLOCKS:
                        bw = nbc * 128
                        lsl = slice(cb0 * 128, cb0 * 128 + bw)
                        ps = psp.tile([128, 512], F32, tag="blk", name="psblk")[:, :bw]
                        for k in range(2):
                            nc.tensor.matmul(ps, wl(OFF_WQ, k, m), xt_like_rhs(xt, k, cb0, nbc),
                                             start=(k == 0), stop=(k == 1))
                        e = escp.tile([128, 512], BF16, tag="esc", name="esc")[:, :bw]
                        nc.scalar.activation(e, ps, AF.Exp)
                        nc.vector.tensor_scalar_min(e, e, 1.0)
                        nc.vector.scalar_tensor_tensor(
                            qt[:, m, lsl], ps, 0.0, e, OP.max, OP.add)

                # ---------- D: den8 / Z / Q^ / msgT ----------
                for cb0, nbc in BLOCKS:
                    bw = nbc * 128
                    lsl = slice(cb0 * 128, cb0 * 128 + bw)
                    psd = psp.tile([8, 512], F32, tag="blk", name="psd8")[:, :bw]
                    for k in range(2):
                        nc.tensor.matmul(psd, a8[:, k, :], qt[:, k, lsl],
                                         start=(k == 0), stop=(k == 1))
                    with nc.allow_low_precision(reason="attention denominator in bf16 is within tolerance"):
                        nc.vector.reciprocal(z8[:, lsl], psd)
                for k in range(2):
                    for cb0, nbc in BLOCKS:
                        bw = nbc * 128
                        lsl = slice(cb0 * 128, cb0 * 128 + bw)
                        psz = psp.tile([128, 512], F32, tag="blk", name="psblk")[:, :bw]
                        nc.tensor.matmul(psz, b8t[:, k, :], z8[:, lsl],
                                         start=True, stop=True)
                        nc.vector.tensor_tensor(qt[:, k, lsl], qt[:, k, lsl], psz, OP.mult)
                msgts = bigp.tile([128, 2, L], BF16, tag="bigE")
                for m in range(2):
                    for cb0, nbc in BLOCKS:
                        bw = nbc * 128
                        lsl = slice(cb0 * 128, cb0 * 128 + bw)
                        psm = psp.tile([128, 512], F32, tag="blk", name="psblk")[:, :bw]
                        nc.tensor.matmul(psm, kvbd[:, m, :], qt[:, m, lsl],
                                         start=True, stop=True)
                        if m == 0:
                            nc.vector.tensor_copy(msgts[:, m, lsl], psm)
                        else:
                            nc.scalar.activation(msgts[:, m, lsl], psm, AF.Copy)

                # ---------- E: Wm + LN1 -> t, tT ----------
                w1t = bigp.tile([128, NCHUNK, 256], BF16, tag="bigC")
                for c in range(NCHUNK):
                    psy = psp.tile([128, 512], F32, tag="blk", name="psy")[:, :257]
                    for k in range(2):
                        nc.tensor.matmul(psy, msgts[:, k, 128 * c:128 * (c + 1)],
                                         wt[:, OFF_WM + k * 257: OFF_WM + (k + 1) * 257],
                                         start=(k == 0), stop=(k == 1))
                    nc.vector.tensor_scalar_sub(w1t[:, c, :], psy[:, :256], psy[:, 256:257])
                    nc.scalar.activation(junk[:], w1t[:, c, :], AF.Square,
                                         accum_out=ss1[:, c:c + 1])
                nc.scalar.activation(lnt[:], ss1[:], AF.Ln, bias=epst[:], scale=1.0 / 256)
                nc.scalar.activation(rs1[:], lnt[:], AF.Exp, scale=-0.5)
                t_ = bigp.tile([128, NCHUNK, 256], BF16, tag="bigA")
                for c in range(NCHUNK):
                    nc.vector.tensor_scalar_mul(t_[:, c, :], w1t[:, c, :], rs1[:, c:c + 1])
                tt = bigp.tile([128, 76, 128], BF16, tag="bigD")
                nc.sync.dma_start_transpose(tt[:], t_[:])

                # ---------- F: h1 (+bias+relu) and h2 + LN2, interleaved ----------
                w2t = bigp.tile([128, NCHUNK, 256], BF16, tag="bigE")
                for cb0, nbc in BLOCKS:
                    bw = nbc * 128
                    h1b = h1p.tile([128, 4, 512], BF16, tag="h1b")
                    for m in range(4):
                        psh = psp.tile([128, 512], F32, tag="blk", name="psblk")[:, :bw]
                        for k in range(2):
                            nc.tensor.matmul(psh, wl(OFF_W1, k, m, per_k=512),
                                             xt_like_rhs(xt, k, cb0, nbc),
                                             start=(k == 0), stop=False)
                        for k in range(2):
                            nc.tensor.matmul(psh, wl(OFF_W1, k + 2, m, per_k=512),
                                             xt_like_rhs(tt, k, cb0, nbc),
                                             start=False, stop=(k == 1))
                        if m < 2:
                            nc.scalar.activation(h1b[:, m, :bw], psh, AF.Relu,
                                                 bias=b1w[:, m:m + 1])
                        else:
                            nc.vector.tensor_scalar(h1b[:, m, :bw], psh, b1w[:, m:m + 1],
                                                    0.0, OP.add, OP.max)
                    for cc in range(nbc):
                        c = cb0 + cc
                        psh2 = psp.tile([128, 512], F32, tag="blk", name="psy")[:, :257]
                        for k in range(4):
                            nc.tensor.matmul(psh2, h1b[:, k, 128 * cc:128 * (cc + 1)],
                                             wt[:, OFF_W2 + k * 257: OFF_W2 + (k + 1) * 257],
                                             start=(k == 0), stop=(k == 3))
                        nc.vector.tensor_scalar_sub(w2t[:, c, :], psh2[:, :256],
                                                    psh2[:, 256:257])
                        nc.vector.scalar_tensor_tensor(
                            junk[:], w2t[:, c, :], 1.0, w2t[:, c, :],
                            OP.bypass, OP.mult, accum_out=ss2[:, c:c + 1])
                nc.scalar.activation(lnt[:], ss2[:], AF.Ln, bias=epst[:], scale=1.0 / 256)
                nc.scalar.activation(rs2[:], lnt[:], AF.Exp, scale=-0.5)
                u = bigp.tile([128, NCHUNK, 256], BF16, tag="bigB")
                for c in range(NCHUNK):
                    nc.vector.tensor_scalar_mul(u[:, c, :], w2t[:, c, :], rs2[:, c:c + 1])
                if apply_g2:
                    gv = g2b2[:, layer, 0, None, :].to_broadcast((128, NCHUNK, 256))
                    nc.vector.tensor_tensor(u[:], u[:], gv, OP.mult)
                if apply_b2:
                    bv = g2b2[:, layer, 1, None, :].to_broadcast((128, NCHUNK, 256))
                    nc.vector.tensor_tensor(u[:], u[:], bv, OP.add)
                nc.vector.tensor_tensor(xn[:], u[:], xn[:], OP.add)
                nc.vector.memset(xn[64:128, NCHUNK - 1, :], 0.0)
                nc.sync.dma_start_transpose(xt[:], xn[:])

            nc.sync.dma_start(y_out.rearrange("p (c d) -> p c d", d=256), xn[:])

    nc.compile()
    return nc


# NOTE on apply_g2/b2: the LN gamma/beta of the second layernorm cannot be
# folded into weights; they are applied with extra DVE passes only when they
# are not the trivial (1, 0).  g1/b1 are always applied exactly (folded into
# W1b / the h1 bias host-side), as are all other parameters.


def _pack_inputs(feat_seq):
    """Host-side packing. feat_seq: [L_REAL, 256] fp32 for one core."""
    bf = ml_dtypes.bfloat16
    x = np.zeros((L, D), np.float32)
    x[:L_REAL] = feat_seq
    xn = np.ascontiguousarray(
        x.reshape(NCHUNK, 128, D).transpose(1, 0, 2).reshape(128, NCHUNK * D)
    ).astype(bf)
    # xt T-form: xt[p, 2c+k, j] = x[c*128+j, k*128+p]
    xr = x.reshape(NCHUNK, 128, 2, 128)          # [c, j, k, p]
    xt = np.ascontiguousarray(
        xr.transpose(3, 0, 2, 1).reshape(128, 2 * L)
    ).astype(bf)
    return xn, xt


def _pack_weights(Wq, Wk, Wv, Wm, W1, W2, g1, b1, g2, b2):
    bf = ml_dtypes.bfloat16
    wts = np.zeros((NLAYERS, 128, WCOLS), np.float32)
    b1w = np.zeros((NLAYERS, 128, 4), np.float32)
    g2b2 = np.zeros((NLAYERS, 2, 256), np.float32)
    for i in range(NLAYERS):
        wq = Wq[i].reshape(2, 128, 256).transpose(1, 0, 2).reshape(128, 512)
        wk = Wk[i].reshape(2, 128, 256).transpose(1, 0, 2).reshape(128, 512)
        wv = Wv[i].reshape(2, 128, 256).transpose(1, 0, 2).reshape(128, 512)
        wm_aug = np.concatenate([Wm[i], Wm[i].mean(axis=1, keepdims=True)], axis=1)
        wm = wm_aug.reshape(2, 128, 257).transpose(1, 0, 2).reshape(128, 514)
        w1cat = np.concatenate([W1[i][:256], g1[i][:, None] * W1[i][256:]], axis=0)
        w1 = w1cat.reshape(4, 128, 512).transpose(1, 0, 2).reshape(128, 2048)
        w2_aug = np.concatenate([W2[i], W2[i].mean(axis=1, keepdims=True)], axis=1)
        w2 = w2_aug.reshape(4, 128, 257).transpose(1, 0, 2).reshape(128, 1028)
        wts[i, :, OFF_WQ:OFF_WQ + 512] = wq
        wts[i, :, OFF_WK:OFF_WK + 512] = wk
        wts[i, :, OFF_WV:OFF_WV + 512] = wv
        wts[i, :, OFF_WM:OFF_WM + 514] = wm
        wts[i, :, OFF_W1:OFF_W1 + 2048] = w1
        wts[i, :, OFF_W2:OFF_W2 + 1028] = w2
        b1w[i] = (b1[i] @ W1[i][256:]).reshape(4, 128).T
        g2b2[i, 0] = g2[i]
        g2b2[i, 1] = b2[i]
    b8 = np.zeros((8, 256), np.float32)
    for k in range(2):
        for j in range(128):
            b8[4 * k + j // 32, k * 128 + j] = 1.0
    ones_t = np.zeros((128, 2), np.float32)
    ones_t[:, 0] = 1.0
    ones_t[:64, 1] = 1.0
    return (wts.astype(bf), b1w, g2b2, b8.astype(bf), ones_t.astype(bf))


class _Runner:
    """Builds the module once and keeps a cached jitted executable
    (mirrors bass2jax.run_bass_via_pjrt's multi-core path, without donation
    so device-resident inputs can be reused across timed iterations)."""

    def __init__(self, ncores, apply_g2, apply_b2):
        import jax
        from jax.sharding import Mesh, PartitionSpec
        from jax.experimental.shard_map import shard_map
        from concourse import bass2jax

        self.ncores = ncores
        nc = _build_module(ncores, apply_g2, apply_b2)
        self.nc = nc
        bass2jax.install_neuronx_cc_hook()

        part_name = nc.partition_id_tensor.name if nc.partition_id_tensor else None
        in_names = []
        out_names = []
        out_avals = []
        zero_outs = []
        for alloc in nc.m.functions[0].allocations:
            if not isinstance(alloc, mybir.MemoryLocationSet):
                continue
            name = alloc.memorylocations[0].name
            if alloc.kind == "ExternalInput":
                if name != part_name:
                    in_names.append(name)
            elif alloc.kind == "ExternalOutput":
                out_names.append(name)
                shape = tuple(alloc.tensor_shape)
                dtype = mybir.dt.np(alloc.dtype)
                out_avals.append(jax.core.ShapedArray(shape, dtype))
                zero_outs.append(np.zeros(shape, dtype))
        self.in_names = in_names
        self.out_names = out_names
        n_params = len(in_names)
        all_names = in_names + out_names
        if part_name is not None:
            all_names = all_names + [part_name]

        def _body(*args):
            operands = list(args)
            if part_name is not None:
                operands.append(bass2jax.partition_id_tensor())
            outs = bass2jax._bass_exec_p.bind(
                *operands,
                out_avals=tuple(out_avals),
                in_names=tuple(all_names),
                out_names=tuple(out_names),
                lowering_input_output_aliases=(),
                sim_require_finite=False,
                sim_require_nnan=False,
                nc=nc,
            )
            return tuple(outs)

        devices = jax.devices()[:ncores]
        assert len(devices) == ncores
        mesh = Mesh(np.asarray(devices), ("core",))
        self.mesh = mesh
        nin = n_params + len(zero_outs)
        self.sharded = jax.jit(
            shard_map(_body, mesh=mesh,
                      in_specs=(PartitionSpec("core"),) * nin,
                      out_specs=(PartitionSpec("core"),) * len(out_names),
                      check_rep=False),
            keep_unused=True,
        )
        self.zero_outs = zero_outs
        self.jax = jax

    def concat_inputs(self, in_maps):
        outs = [
            np.concatenate([np.asarray(m[name]) for m in in_maps], axis=0)
            for name in self.in_names
        ]
        outs += [
            np.zeros((self.ncores * z.shape[0], *z.shape[1:]), z.dtype)
            for z in self.zero_outs
        ]
        return outs

    def run(self, in_maps):
        args = self.concat_inputs(in_maps)
        out_arrs = self.sharded(*args)
        res = []
        for c in range(self.ncores):
            d = {}
            for i, name in enumerate(self.out_names):
                full = np.asarray(out_arrs[i])
                d[name] = full.reshape(
                    self.ncores, full.shape[0] // self.ncores, *full.shape[1:])[c]
            res.append(d)
        return res


_cache = {}


def _get_runner(ncores, apply_g2, apply_b2):
    key = (ncores, apply_g2, apply_b2)
    if key not in _cache:
        _cache[key] = _Runner(ncores, apply_g2, apply_b2)
    return _cache[key]


_wcache = {}


def prepare(feat0, feat1, Wq, Wk, Wv, Wm, W1, W2, g1, b1, g2, b2):
    """Pack inputs and return (runner, in_maps) — shared by kernel() and test.py."""
    feat0 = np.asarray(feat0, np.float32)
    feat1 = np.asarray(feat1, np.float32)
    N = feat0.shape[0]
    assert N * 2 == N_CORES

    wkey = id(Wq)
    if wkey not in _wcache:
        _wcache.clear()
        args = [np.asarray(a, np.float32) for a in
                (Wq, Wk, Wv, Wm, W1, W2, g1, b1, g2, b2)]
        packed = _pack_weights(*args)
        apply_g2 = not np.allclose(args[8], 1.0)
        apply_b2 = not np.allclose(args[9], 0.0)
        _wcache[wkey] = (packed, apply_g2, apply_b2)
    (wts, b1w, g2b2, b8, ones_t), apply_g2, apply_b2 = _wcache[wkey]

    runner = _get_runner(N_CORES, apply_g2, apply_b2)

    g2b2_rep = None
    if apply_g2 or apply_b2:
        g2b2_rep = np.ascontiguousarray(np.broadcast_to(
            g2b2.reshape(1, NLAYERS * 2 * 256), (128, NLAYERS * 2 * 256)))

    in_maps = []
    for core in range(N_CORES):
        seq = feat0[core // 2] if core % 2 == 0 else feat1[core // 2]
        xn, xt = _pack_inputs(seq)
        m = dict(xn_in=xn, xt_in=xt, wts=wts, b1w=b1w, b8=b8, ones_t=ones_t)
        if g2b2_rep is not None:
            m["g2b2"] = g2b2_rep
        in_maps.append(m)
    return runner, in_maps


def kernel(feat0, feat1, Wq, Wk, Wv, Wm, W1, W2, g1, b1, g2, b2):
    feat0 = np.asarray(feat0, np.float32)
    feat1 = np.asarray(feat1, np.float32)
    N = feat0.shape[0]
    runner, in_maps = prepare(feat0, feat1, Wq, Wk, Wv, Wm, W1, W2,
                              g1, b1, g2, b2)
    res = runner.run(in_maps)

    out0 = np.empty((N, L_REAL, D), np.float32)
    out1 = np.empty((N, L_REAL, D), np.float32)
    for core in range(N_CORES):
        y = np.asarray(res[core]["y"], np.float32)  # [128, NCHUNK*256]
        xfull = y.reshape(128, NCHUNK, D).transpose(1, 0, 2).reshape(L, D)[:L_REAL]
        if core % 2 == 0:
            out0[core // 2] = xfull
        else:
            out1[core // 2] = xfull
    return out0, out1
